# revision 1
# baseline (speedup 1.0000x reference)
"""Trainium2 Bass kernel for nn_ItemAutoencoder (LSTM autoencoder).

Model: x[B,T,D] -> relu(x @ in_W.T + in_b)            [B,T,64]
         -> LSTM(64->256) -> LSTM(256->256)            [B,T,256]
         -> z = h[:, -1]                               [B,256]
         -> repeat z over T -> LSTM(256->64) -> LSTM(64->64)
         -> out = d @ out_W.T + out_b                  [B,T,256]
B=1024, T=100, D=256.  Sharding: data-parallel, batch 128 per core x 8 cores.

Per-core layout strategy:
  - All recurrent state h is kept TRANSPOSED ([H, B] packed as [128, (H/128)*128]
    SBUF tiles) so it can be used directly as the matmul stationary operand
    (lhsT) of the gate matmuls: gates[B, 4H] = hT.T @ W.T.
  - Gate order is host-permuted to [f, i, o, g] so sigmoid/tanh cover
    contiguous column ranges, and f completes first (it heads the recurrence
    critical path c' = f*c + i*g).
  - Biases ride the matmuls: the in-proj output carries a constant "ones" row
    (so enc-L0's K=65 input matmul adds the bias), enc-L1 uses a K=1 ones
    vector x bias row matmul, the decoder input matmuls use ones-row-augmented
    state tiles, and dec-L0 re-injects its (constant-over-time) input
    contribution each step via an identity-matmul from SBUF.
  - Matmuls run in float32r (full PE rate at moving-dim >= 256); elementwise
    and activations in fp32.
"""
import os
import numpy as np
from contextlib import ExitStack

import concourse.bass as bass
import concourse.tile as tile
from concourse import bacc, mybir
from concourse import bass_utils

F32 = mybir.dt.float32
F32R = mybir.dt.float32r
BF16 = mybir.dt.bfloat16
import ml_dtypes
MM_DT = BF16 if os.environ.get("KERNEL_MM_DT", "bf16") == "bf16" else F32R
MM_NP = ml_dtypes.bfloat16 if MM_DT == BF16 else np.float32
AF = mybir.ActivationFunctionType
TS = bass.ts

N_CORES = 8
B = 128            # per-core batch
T = 100
D = 256
H = 256            # encoder hidden
M = 64             # in-proj dim / decoder hidden
GE = 4 * H         # 1024
GD = 4 * M         # 256

_CACHE: dict = {}


# ----------------------------------------------------------------------------
# kernel builder (per-core program; SPMD across 8 cores)
# ----------------------------------------------------------------------------

def build_nc():
    nc = bacc.Bacc("TRN2", target_bir_lowering=False, debug=False)

    # ---- DRAM I/O -----------------------------------------------------------
    xT_d = nc.dram_tensor("xT", [2, 128, T * B], MM_DT, kind="ExternalInput")
    inWT_d = nc.dram_tensor("inWT", [2, 128, M], MM_DT, kind="ExternalInput")
    inb_d = nc.dram_tensor("inb", [M, 1], F32, kind="ExternalInput")
    w0in_d = nc.dram_tensor("w0in", [M + 1, GE], MM_DT, kind="ExternalInput")
    w0rec_d = nc.dram_tensor("w0rec", [2, 128, GE], MM_DT, kind="ExternalInput")
    w1in_d = nc.dram_tensor("w1in", [2, 128, GE], MM_DT, kind="ExternalInput")
    w1rec_d = nc.dram_tensor("w1rec", [2, 128, GE], MM_DT, kind="ExternalInput")
    b1_d = nc.dram_tensor("b1", [1, GE], MM_DT, kind="ExternalInput")
    dw0in_d = nc.dram_tensor("dw0in", [2, 128, GD], MM_DT, kind="ExternalInput")
    bd0_d = nc.dram_tensor("bd0", [1, GD], MM_DT, kind="ExternalInput")
    dw0rec_d = nc.dram_tensor("dw0rec", [M, GD], MM_DT, kind="ExternalInput")
    dw1in_d = nc.dram_tensor("dw1in", [M + 1, GD], MM_DT, kind="ExternalInput")
    dw1rec_d = nc.dram_tensor("dw1rec", [M, GD], MM_DT, kind="ExternalInput")
    wout_d = nc.dram_tensor("wout", [M + 1, D], MM_DT, kind="ExternalInput")
    ident_d = nc.dram_tensor("ident", [128, 128], F32, kind="ExternalInput")
    identr_d = nc.dram_tensor("identr", [128, 128], MM_DT, kind="ExternalInput")
    ones1_d = nc.dram_tensor("ones1", [1, 128], MM_DT, kind="ExternalInput")
    out_d = nc.dram_tensor("out", [B, T * D], F32, kind="ExternalOutput")

    with tile.TileContext(nc) as tc, ExitStack() as ctx:
        P = bass.MemorySpace.PSUM
        wp = ctx.enter_context(tc.tile_pool(name="w", bufs=1))

        def wtile(dram_ap, shape, tag, dt=MM_DT):
            t_ = wp.tile(shape, dt, tag=tag)
            nc.sync.dma_start(t_[:], dram_ap)
            return t_

        # ---- persistent weights in SBUF ------------------------------------
        inWT = [wtile(inWT_d[kb, :, :], [128, M], f"inWT{kb}") for kb in range(2)]
        inb = wtile(inb_d[:], [M, 1], "inb", F32)
        w0in = wtile(w0in_d[:], [M + 1, GE], "w0in")
        w0rec = [wtile(w0rec_d[kb, :, :], [128, GE], f"w0rec{kb}") for kb in range(2)]
        w1in = [wtile(w1in_d[kb, :, :], [128, GE], f"w1in{kb}") for kb in range(2)]
        w1rec = [wtile(w1rec_d[kb, :, :], [128, GE], f"w1rec{kb}") for kb in range(2)]
        b1 = wtile(b1_d[:], [1, GE], "b1")
        dw0in = [wtile(dw0in_d[kb, :, :], [128, GD], f"dw0in{kb}") for kb in range(2)]
        bd0 = wtile(bd0_d[:], [1, GD], "bd0")
        dw0rec = wtile(dw0rec_d[:], [M, GD], "dw0rec")
        dw1in = wtile(dw1in_d[:], [M + 1, GD], "dw1in")
        dw1rec = wtile(dw1rec_d[:], [M, GD], "dw1rec")
        wout = wtile(wout_d[:], [M + 1, D], "wout")
        ident = wtile(ident_d[:], [128, 128], "ident", F32)
        identr = wtile(identr_d[:], [128, 128], "identr")
        ones1 = wtile(ones1_d[:], [1, 128], "ones1")

        # in-proj output, transposed, with a ones row (row 64) for bias riding
        h0aug_h = nc.alloc_sbuf_tensor("h0aug", [M + 1, T * B], MM_DT)
        h0aug = h0aug_h.ap()
        if MM_DT == F32R:
            nc.gpsimd.memset(h0aug[M : M + 1, :].bitcast(F32), 1.0)
        else:
            nc.gpsimd.memset(h0aug[M : M + 1, :], 1.0)

        # ============================= in-proj ==============================
        # h0T[m, (t,b)] = relu(in_W @ x_t.T + in_b), computed in groups of 4 t
        xpool = ctx.enter_context(tc.tile_pool(name="xc", bufs=4))
        with tc.tile_pool(name="psip", bufs=2, space=P) as psip:
            NG = T * B // 512  # 25
            for g in range(NG):
                xa = xpool.tile([128, 512], MM_DT, tag="xa")
                xb = xpool.tile([128, 512], MM_DT, tag="xb")
                nc.sync.dma_start(xa[:], xT_d[0, :, TS(g, 512)])
                nc.sync.dma_start(xb[:], xT_d[1, :, TS(g, 512)])
                ps = psip.tile([M, 512], F32)
                nc.tensor.matmul(ps[:], inWT[0][:], xa[:], start=True, stop=False)
                nc.tensor.matmul(ps[:], inWT[1][:], xb[:], start=False, stop=True)
                nc.scalar.activation(
                    h0aug[0:M, TS(g, 512)], ps[:], AF.Relu, bias=inb[:, 0:1]
                )

        # ============================= encoder ==============================
        # layer params: (in_srcs(t) -> list of (lhsT_ap, rhs_tile), rec weights)
        gpool = ctx.enter_context(tc.tile_pool(name="g", bufs=2))
        spool = ctx.enter_context(tc.tile_pool(name="s", bufs=2))
        hpool = ctx.enter_context(tc.tile_pool(name="h", bufs=3))

        def new_state(tag, shape, dt, pool, zero=True):
            t_ = pool.tile(shape, dt, tag=tag)
            if zero:
                nc.gpsimd.memset(t_[:].bitcast(F32) if dt == F32R else t_[:], 0.0)
            return t_

        st = {
            0: {
                "hT": new_state("h0T", [128, H], MM_DT, hpool),
                "c": new_state("c0", [128, H], F32, spool),
            },
            1: {
                "hT": new_state("h1T", [128, H], MM_DT, hpool),
                "c": new_state("c1", [128, H], F32, spool),
            },
        }

        with (
            tc.tile_pool(name="pg0", bufs=1, space=P) as pg0p,
            tc.tile_pool(name="pg1", bufs=1, space=P) as pg1p,
            tc.tile_pool(name="ptr", bufs=2, space=P) as ptrp,
        ):
            pgp = {0: pg0p, 1: pg1p}

            def enc_mms(l, t):
                """Emit gate matmuls for encoder layer l at step t."""
                s = st[l]
                hT = s["hT"]
                if l == 0:
                    srcs = [(h0aug[:, TS(t, 128)], w0in)]
                else:
                    srcs = [
                        (ones1[:], b1),
                        (s["in_hT"][:, 0:128], w1in[0]),
                        (s["in_hT"][:, 128:256], w1in[1]),
                    ]
                wrec = w0rec if l == 0 else w1rec
                srcs += [
                    (hT[:, 0:128], wrec[0]),
                    (hT[:, 128:256], wrec[1]),
                ]
                ps = pgp[l].tile([128, GE], F32, tag=f"pg{l}")
                s["ps"] = ps
                for chunk in range(2):
                    ns = slice(chunk * 512, (chunk + 1) * 512)
                    for j, (lh, rh) in enumerate(srcs):
                        nc.tensor.matmul(
                            ps[:, ns],
                            lh,
                            rh[:, ns],
                            start=(j == 0),
                            stop=(j == len(srcs) - 1),
                        )

            def enc_act_gates(l):
                s = st[l]
                ps = s["ps"]
                gsb = gpool.tile([128, GE], F32, tag=f"gsb{l}")
                s["gsb"] = gsb
                # layout [f, i, o, g]; chunk0 = f,i ; chunk1 = o,g
                nc.scalar.activation(gsb[:, 0:H], ps[:, 0:H], AF.Sigmoid)
                nc.scalar.activation(gsb[:, H : 3 * H], ps[:, H : 3 * H], AF.Sigmoid)
                nc.scalar.activation(gsb[:, 3 * H :], ps[:, 3 * H :], AF.Tanh)

            def enc_dve_c(l):
                s = st[l]
                gsb = s["gsb"]
                ctmp = spool.tile([128, H], F32, tag=f"ctmp{l}")
                u = spool.tile([128, H], F32, tag=f"u{l}")
                c_new = spool.tile([128, H], F32, tag=f"c{l}")
                nc.vector.tensor_mul(ctmp[:], gsb[:, 0:H], s["c"][:])
                nc.vector.tensor_mul(u[:], gsb[:, H : 2 * H], gsb[:, 3 * H :])
                nc.vector.tensor_add(c_new[:], ctmp[:], u[:])
                s["c"] = c_new

            def enc_act_tanhc(l):
                s = st[l]
                tcs = spool.tile([128, H], F32, tag=f"tc{l}")
                nc.scalar.activation(tcs[:], s["c"][:], AF.Tanh)
                s["tc"] = tcs

            def enc_dve_h(l):
                s = st[l]
                hsb = spool.tile([128, H], F32, tag=f"hsb{l}")
                nc.vector.tensor_mul(hsb[:], s["gsb"][:, 2 * H : 3 * H], s["tc"][:])
                s["h"] = hsb

            def enc_pe_tr(l):
                s = st[l]
                ptr = ptrp.tile([128, H], F32, tag="ptr")
                nc.tensor.transpose(ptr[:, 0:128], s["h"][:, 0:128], ident[:])
                nc.tensor.transpose(ptr[:, 128:256], s["h"][:, 128:256], ident[:])
                s["ptr"] = ptr

            def enc_copy_h(l):
                s = st[l]
                hT_new = hpool.tile([128, H], MM_DT, tag=f"h{l}T")
                if l == 0:
                    nc.scalar.activation(hT_new[:], s["ptr"][:], AF.Copy)
                else:
                    nc.vector.tensor_copy(hT_new[:], s["ptr"][:])
                s["hT"] = hT_new

            def enc_slot(work):  # work = list of (layer, t)
                for l, t in work:
                    enc_mms(l, t)
                for l, _ in work:
                    enc_act_gates(l)
                for l, _ in work:
                    enc_dve_c(l)
                for l, _ in work:
                    enc_act_tanhc(l)
                for l, _ in work:
                    enc_dve_h(l)
                for l, _ in work:
                    enc_pe_tr(l)
                for l, _ in work:
                    enc_copy_h(l)

            for t in range(T):
                work = [(0, t)]
                if t >= 1:
                    # L1 consumes h0T(t-1), which is st[0]["hT"] *before* L0(t)
                    # updates it -> snapshot ref now
                    st[1]["in_hT"] = st[0]["hT"]
                    work.append((1, t - 1))
                    # reorder: L0 first then L1 (emission order = engine order)
                enc_slot(work)
            st[1]["in_hT"] = st[0]["hT"]
            enc_slot([(1, T - 1)])

        zT = st[1]["hT"]  # [128, 256] fp32r = h1T(T-1) packed

        # ============================= decoder ==============================
        dst = {}
        with (
            tc.tile_pool(name="pd0", bufs=1, space=P) as pd0p,
            tc.tile_pool(name="pd1", bufs=1, space=P) as pd1p,
            tc.tile_pool(name="ptrd", bufs=2, space=P) as ptrdp,
            tc.tile_pool(name="pout", bufs=2, space=P) as poutp,
            tc.tile_pool(name="pxg", bufs=1, space=P) as pxgp,
        ):
            # xg0 = z @ dW0i.T + bd0  (constant over time)
            psx = pxgp.tile([128, GD], F32)
            nc.tensor.matmul(psx[:], ones1[:], bd0[:], start=True, stop=False)
            nc.tensor.matmul(psx[:], zT[:, 0:128], dw0in[0][:], start=False, stop=False)
            nc.tensor.matmul(psx[:], zT[:, 128:256], dw0in[1][:], start=False, stop=True)
            xg0 = wp.tile([128, GD], MM_DT, tag="xg0")
            nc.scalar.activation(xg0[:], psx[:], AF.Copy)

            # persistent ones-row-augmented transposed decoder states
            def dstate(tag):
                t_ = nc.alloc_sbuf_tensor(tag, [M + 1, 128], MM_DT).ap()
                if MM_DT == F32R:
                    nc.gpsimd.memset(t_[:].bitcast(F32), 0.0)
                    nc.gpsimd.memset(t_[M : M + 1, :].bitcast(F32), 1.0)
                else:
                    nc.gpsimd.memset(t_[:], 0.0)
                    nc.gpsimd.memset(t_[M : M + 1, :], 1.0)
                return t_

            for l in range(2):
                dst[l] = {
                    "dT": [dstate(f"d{l}A"), dstate(f"d{l}B"), dstate(f"d{l}Z")],
                    "c": new_state(f"cd{l}", [128, M], F32, spool),
                    "prev": 2,  # index of zero-init tile
                }

            pdp = {0: pd0p, 1: pd1p}

            def dec_mms(l, t):
                s = dst[l]
                dT_prev = s["dT"][s["prev"]]
                ps = pdp[l].tile([128, GD], F32, tag=f"pd{l}")
                s["ps"] = ps
                if l == 0:
                    nc.tensor.matmul(ps[:], identr[:], xg0[:], start=True, stop=False)
                    nc.tensor.matmul(
                        ps[:], dT_prev[0:M, :], dw0rec[:], start=False, stop=True
                    )
                else:
                    d0T = dst[0]["in_dT"]
                    nc.tensor.matmul(ps[:], d0T[0 : M + 1, :], dw1in[:], start=True, stop=False)
                    nc.tensor.matmul(
                        ps[:], dT_prev[0:M, :], dw1rec[:], start=False, stop=True
                    )

            def dec_act_gates(l):
                s = dst[l]
                ps = s["ps"]
                gsb = gpool.tile([128, GD], F32, tag=f"dgsb{l}")
                s["gsb"] = gsb
                nc.scalar.activation(gsb[:, 0:M], ps[:, 0:M], AF.Sigmoid)
                nc.scalar.activation(gsb[:, M : 3 * M], ps[:, M : 3 * M], AF.Sigmoid)
                nc.scalar.activation(gsb[:, 3 * M :], ps[:, 3 * M :], AF.Tanh)

            def dec_dve_c(l):
                s = dst[l]
                gsb = s["gsb"]
                ctmp = spool.tile([128, M], F32, tag=f"dctmp{l}")
                u = spool.tile([128, M], F32, tag=f"du{l}")
                c_new = spool.tile([128, M], F32, tag=f"dc{l}")
                nc.vector.tensor_mul(ctmp[:], gsb[:, 0:M], s["c"][:])
                nc.vector.tensor_mul(u[:], gsb[:, M : 2 * M], gsb[:, 3 * M :])
                nc.vector.tensor_add(c_new[:], ctmp[:], u[:])
                s["c"] = c_new

            def dec_act_tanhc(l):
                s = dst[l]
                tcs = spool.tile([128, M], F32, tag=f"dtc{l}")
                nc.scalar.activation(tcs[:], s["c"][:], AF.Tanh)
                s["tc"] = tcs

            def dec_dve_h(l):
                s = dst[l]
                hsb = spool.tile([128, M], F32, tag=f"dhsb{l}")
                nc.vector.tensor_mul(hsb[:], s["gsb"][:, 2 * M : 3 * M], s["tc"][:])
                s["h"] = hsb

            def dec_pe_tr(l, t):
                s = dst[l]
                ptr = ptrdp.tile([M, 128], F32, tag="ptrd")
                nc.tensor.transpose(ptr[:], s["h"][:], ident[:])
                s["ptr"] = ptr

            def dec_copy_h(l, t):
                s = dst[l]
                dT_new = s["dT"][t % 2]
                if l == 0:
                    nc.scalar.activation(dT_new[0:M, :], s["ptr"][:], AF.Copy)
                else:
                    nc.vector.tensor_copy(dT_new[0:M, :], s["ptr"][:])
                s["prev"] = t % 2

            # out-proj state
            ochunk = {"tile": None}

            def outproj_mm(t):
                d1T = dst[1]["dT"][t % 2]
                ps = poutp.tile([128, D], F32, tag="pout")
                nc.tensor.matmul(ps[:], d1T[0 : M + 1, :], wout[:], start=True, stop=True)
                return ps

            def outproj_copy(t, ps):
                if t % 10 == 0:
                    ochunk["tile"] = spool.tile(
                        [128, 10 * D], F32, tag="ochunk", name="ochunk"
                    )
                dest = ochunk["tile"][:, TS(t % 10, D)]
                if t % 2 == 0:
                    nc.scalar.activation(dest, ps[:], AF.Copy)
                else:
                    nc.vector.tensor_copy(dest, ps[:])
                if t % 10 == 9:
                    nc.sync.dma_start(out_d[:, TS(t // 10, 10 * D)], ochunk["tile"][:])

            def dec_slot(work, op_t):
                for l, t in work:
                    dec_mms(l, t)
                ps_out = outproj_mm(op_t) if op_t is not None else None
                for l, _ in work:
                    dec_act_gates(l)
                for l, _ in work:
                    dec_dve_c(l)
                for l, _ in work:
                    dec_act_tanhc(l)
                for l, _ in work:
                    dec_dve_h(l)
                if op_t is not None:
                    outproj_copy(op_t, ps_out)
                for l, t in work:
                    dec_pe_tr(l, t)
                for l, t in work:
                    dec_copy_h(l, t)

            for t in range(T):
                work = [(0, t)]
                if t >= 1:
                    dst[0]["in_dT"] = dst[0]["dT"][(t - 1) % 2]
                    work.append((1, t - 1))
                op_t = t - 2 if t >= 2 else None
                dec_slot(work, op_t)
            dst[0]["in_dT"] = dst[0]["dT"][(T - 1) % 2]
            dec_slot([(1, T - 1)], T - 2)
            dec_slot([], T - 1)

    nc.compile()
    return nc


# ----------------------------------------------------------------------------
# host-side wrapper
# ----------------------------------------------------------------------------

def _perm(n):
    """pytorch gate order i,f,g,o (blocks of n) -> [f, i, o, g]."""
    idx = np.arange(4 * n).reshape(4, n)
    return np.concatenate([idx[1], idx[0], idx[3], idx[2]])


def _prep_core_inputs(inputs, core):
    """Build the per-core in_map (numpy layout prep only)."""
    f = np.float32
    pe = _perm(H)
    pd = _perm(M)
    x = inputs["x"][core * B : (core + 1) * B]          # [128, 100, 256]
    xT = np.ascontiguousarray(x.transpose(2, 1, 0)).reshape(2, 128, T * B)

    w0in = np.concatenate(
        [inputs["eW0i"].T[:, pe], (inputs["eb0i"] + inputs["eb0h"])[None, pe]], 0
    )
    w0rec = inputs["eW0h"].T[:, pe].reshape(2, 128, GE)
    w1in = inputs["eW1i"].T[:, pe].reshape(2, 128, GE)
    w1rec = inputs["eW1h"].T[:, pe].reshape(2, 128, GE)
    b1 = (inputs["eb1i"] + inputs["eb1h"])[None, pe]
    dw0in = inputs["dW0i"].T[:, pd].reshape(2, 128, GD)
    bd0 = (inputs["db0i"] + inputs["db0h"])[None, pd]
    dw0rec = inputs["dW0h"].T[:, pd]
    dw1in = np.concatenate(
        [inputs["dW1i"].T[:, pd], (inputs["db1i"] + inputs["db1h"])[None, pd]], 0
    )
    dw1rec = inputs["dW1h"].T[:, pd]
    wout = np.concatenate([inputs["out_W"].T, inputs["out_b"][None, :]], 0)

    g = MM_NP
    return {
        "xT": np.ascontiguousarray(xT, dtype=g),
        "inWT": np.ascontiguousarray(inputs["in_W"].T.reshape(2, 128, M), dtype=g),
        "inb": np.ascontiguousarray(inputs["in_b"][:, None], dtype=f),
        "w0in": np.ascontiguousarray(w0in, dtype=g),
        "w0rec": np.ascontiguousarray(w0rec, dtype=g),
        "w1in": np.ascontiguousarray(w1in, dtype=g),
        "w1rec": np.ascontiguousarray(w1rec, dtype=g),
        "b1": np.ascontiguousarray(b1, dtype=g),
        "dw0in": np.ascontiguousarray(dw0in, dtype=g),
        "bd0": np.ascontiguousarray(bd0, dtype=g),
        "dw0rec": np.ascontiguousarray(dw0rec, dtype=g),
        "dw1in": np.ascontiguousarray(dw1in, dtype=g),
        "dw1rec": np.ascontiguousarray(dw1rec, dtype=g),
        "wout": np.ascontiguousarray(wout, dtype=g),
        "ident": np.eye(128, dtype=f),
        "identr": np.eye(128).astype(g),
        "ones1": np.ones((1, 128), dtype=g),
    }


def kernel(**inputs):
    inputs = {k: np.asarray(v, dtype=np.float32) for k, v in inputs.items()}
    if "nc" not in _CACHE:
        _CACHE["nc"] = build_nc()
    nc = _CACHE["nc"]
    in_maps = [_prep_core_inputs(inputs, c) for c in range(N_CORES)]
    trace = bool(int(os.environ.get("KERNEL_TRACE", "0")))
    res = bass_utils.run_bass_kernel_spmd(
        nc,
        in_maps,
        core_ids=list(range(N_CORES)),
        trace=trace,
        tmpdir=os.environ.get("KERNEL_TRACE_DIR") or None,
    )
    _CACHE["last_result"] = res
    out = np.concatenate(
        [res.results[c]["out"].reshape(B, T, D) for c in range(N_CORES)], axis=0
    )
    return out

